# revision 4
# baseline (speedup 1.0000x reference)
"""TRN2 Bass kernel for nn_CSCOut_61503931678830 (e3nn-style gated tensor product).

Self-contained: computes Wigner-3j constants locally, shards the atom dim
across 8 NeuronCores (pure data parallel), and runs one Bass/Tile kernel
per core via run_bass_kernel_spmd.

Math (validated vs reference in fp32):
  paths 3,4 (antisymmetric C111/C221 with x1==x2) are identically zero.
  res[n] = F[n] @ M where F = [out0(1), B5(5), G9(9), H25(25)] and all
  path norms / C-diagonals / Q-basis / axis permutation fold into M and
  the preprocessed weights.
"""
import math
import os
import sys
import time

sys.path.insert(0, "/opt/trn_rl_repo")

import numpy as np

# ----------------------------------------------------------------------------
# Wigner-3j constants (identical math to the reference, self-contained)
# ----------------------------------------------------------------------------

def _cg(j1, m1, j2, m2, j3, m3):
    if m3 != m1 + m2:
        return 0.0
    f = math.factorial
    vmin = max(-j1 + j2 + m3, -j1 + m1, 0)
    vmax = min(j2 + j3 + m1, j3 - j1 + j2, j3 + m3)
    pref = math.sqrt((2 * j3 + 1) * f(j3 + j1 - j2) * f(j3 - j1 + j2) * f(j1 + j2 - j3) / f(j1 + j2 + j3 + 1))
    pref *= math.sqrt(f(j3 + m3) * f(j3 - m3) / (f(j1 + m1) * f(j1 - m1) * f(j2 + m2) * f(j2 - m2)))
    s = 0.0
    for v in range(vmin, vmax + 1):
        s += (-1.0) ** (v + j2 + m2) * f(j2 + j3 + m1 - v) * f(j1 - m1 + v) / (
            f(v) * f(j3 - j1 + j2 - v) * f(j3 + m3 - v) * f(v + j1 - j2 - m3))
    return pref * s


def _r2c(l):
    q = np.zeros((2 * l + 1, 2 * l + 1), dtype=np.complex128)
    for m in range(-l, 0):
        q[l + m, l + abs(m)] = 1 / 2 ** 0.5
        q[l + m, l - abs(m)] = -1j / 2 ** 0.5
    q[l, l] = 1.0
    for m in range(1, l + 1):
        q[l + m, l + abs(m)] = (-1) ** m / 2 ** 0.5
        q[l + m, l - abs(m)] = 1j * (-1) ** m / 2 ** 0.5
    return (-1j) ** l * q


def _w3j(l1, l2, l3):
    C = np.zeros((2 * l1 + 1, 2 * l2 + 1, 2 * l3 + 1))
    for m1 in range(-l1, l1 + 1):
        for m2 in range(-l2, l2 + 1):
            m3 = m1 + m2
            if abs(m3) <= l3:
                C[l1 + m1, l2 + m2, l3 + m3] = _cg(l1, m1, l2, m2, l3, m3)
    C = C / math.sqrt(2 * l3 + 1)
    Cr = np.einsum('ij,kl,mn,ikm->jln', _r2c(l1), _r2c(l2), _r2c(l3), C.astype(np.complex128))
    Cr = Cr.imag if np.linalg.norm(Cr.imag) > np.linalg.norm(Cr.real) else Cr.real
    return (Cr / np.linalg.norm(Cr)).astype(np.float32)


_C110, _C220 = _w3j(1, 1, 0), _w3j(2, 2, 0)
_C022, _C112, _C222 = _w3j(0, 2, 2), _w3j(1, 1, 2), _w3j(2, 2, 2)
_Q = np.concatenate([math.sqrt(2 * l + 1) * np.transpose(_w3j(1, 1, l), (2, 0, 1)) for l in (0, 1, 2)], axis=0)
_N0, _N2 = (1.0 / 48.0) ** 0.5, (5.0 / 64.0) ** 0.5
_idx = [2, 0, 1]
_Qp = _Q[:, _idx, :][:, :, _idx]
_c110 = float(_C110[0, 0, 0])
_c220 = float(_C220[0, 0, 0])
_sig = np.array([_C022[0, k, k] for k in range(5)], np.float64)

N_CORES = 8
N_FULL = 200_000
A_CORE = 25_088          # 49 * 512, padded per-core atom count
N_PAD = N_CORES * A_CORE
N_MACRO = int(os.environ.get("KERNEL_MACROS", A_CORE // 512))  # 49


def _host_prep(W_lin0, W_lin1, W_lin2, W_mlp1, b_mlp1, W_mlp2, b_mlp2):
    Ws = (W_lin0 / math.sqrt(128.0)).astype(np.float32)          # [128,16]
    Wv = np.zeros((192, 48), np.float32)
    for i in range(3):
        Wv[i::3, i * 16:(i + 1) * 16] = W_lin1 / math.sqrt(64.0)
    Wt = np.zeros((160, 80), np.float32)
    for k in range(5):
        Wt[k::5, k * 16:(k + 1) * 16] = W_lin2 / math.sqrt(32.0)
    W2c = np.zeros((64, 96), np.float32)
    b2c = np.zeros((1, 96), np.float32)
    scales = [_N0, _N0 * _c110, _N0 * _c220, None, _N2, _N2]
    blocks = [0, 1, 2, None, 6, 8]
    for j, (p, a) in enumerate(zip(blocks, scales)):
        if p is None:
            W2c[:, j * 16:(j + 1) * 16] = _N2 * (W_mlp2[:, 80:96] + W_mlp2[:, 112:128])
            b2c[0, j * 16:(j + 1) * 16] = _N2 * (b_mlp2[80:96] + b_mlp2[112:128])
        else:
            W2c[:, j * 16:(j + 1) * 16] = a * W_mlp2[:, p * 16:(p + 1) * 16]
            b2c[0, j * 16:(j + 1) * 16] = a * b_mlp2[p * 16:(p + 1) * 16]
    # final map M [40, 9] -> symmetric 6 cols [xx,yy,zz,xy,xz,yz]
    M9 = np.zeros((40, 9), np.float64)
    M9[0] = _Qp[0].reshape(9)
    for k in range(5):
        M9[1 + k] = _sig[k] * _Qp[4 + k].reshape(9)
    for i in range(3):
        for j in range(3):
            for k in range(5):
                M9[6 + i * 3 + j] += _C112[i, j, k] * _Qp[4 + k].reshape(9)
    for i in range(5):
        for j in range(5):
            for k in range(5):
                M9[15 + i * 5 + j] += _C222[i, j, k] * _Qp[4 + k].reshape(9)
    M9 = M9.reshape(40, 3, 3)
    M6 = np.stack([M9[:, 0, 0], M9[:, 1, 1], M9[:, 2, 2],
                   M9[:, 0, 1], M9[:, 0, 2], M9[:, 1, 2]], axis=1)  # [40, 6]
    Mrep = np.tile(M6.T.reshape(1, 240).astype(np.float32), (128, 1))  # [128, 6*40] ab-major
    ones = np.ones((1, 512), np.float32)
    return dict(Ws=Ws, Wva=Wv[:128].copy(), Wvb=Wv[128:].copy(),
                Wta=Wt[:128].copy(), Wtb=Wt[128:].copy(),
                W1=W_mlp1.astype(np.float32), b1=b_mlp1.reshape(1, 64).astype(np.float32),
                W2=W2c, b2=b2c, Mrep=Mrep, ones=ones)


_NC_CACHE = [None]


def _build_nc():
    import concourse.bacc as bacc
    import concourse.tile as tile
    from concourse import mybir
    from concourse.masks import make_identity

    F32 = mybir.dt.float32
    F32R = mybir.dt.float32r
    AX = mybir.AxisListType.X
    ADD = mybir.AluOpType.add
    ACTF = mybir.ActivationFunctionType

    nc = bacc.Bacc("TRN2", target_bir_lowering=False, debug=False)
    X = nc.declare_dram_parameter("X", [A_CORE, 608], F32, isOutput=False)
    Wsd = nc.declare_dram_parameter("Ws", [128, 16], F32, isOutput=False)
    Wvad = nc.declare_dram_parameter("Wva", [128, 48], F32, isOutput=False)
    Wvbd = nc.declare_dram_parameter("Wvb", [64, 48], F32, isOutput=False)
    Wtad = nc.declare_dram_parameter("Wta", [128, 80], F32, isOutput=False)
    Wtbd = nc.declare_dram_parameter("Wtb", [32, 80], F32, isOutput=False)
    W1d = nc.declare_dram_parameter("W1", [128, 64], F32, isOutput=False)
    b1d = nc.declare_dram_parameter("b1", [1, 64], F32, isOutput=False)
    W2d = nc.declare_dram_parameter("W2", [64, 96], F32, isOutput=False)
    b2d = nc.declare_dram_parameter("b2", [1, 96], F32, isOutput=False)
    Md = nc.declare_dram_parameter("Mrep", [128, 240], F32, isOutput=False)
    Od = nc.declare_dram_parameter("ones", [1, 512], F32, isOutput=False)
    Y = nc.declare_dram_parameter("Y", [A_CORE, 9], F32, isOutput=True)

    with tile.TileContext(nc) as tc:
        with (
            tc.tile_pool(name="consts", bufs=1) as consts,
            tc.tile_pool(name="stage", bufs=2) as stage,
            tc.tile_pool(name="ing", bufs=2) as ingp,
            tc.tile_pool(name="tp", bufs=2) as tpp,
            tc.tile_pool(name="ps", bufs=1, space="PSUM") as ps,
        ):
            # ---- one-time weight staging (fp32 load -> f32r rounded) ----
            ident = consts.tile([128, 128], F32)
            make_identity(nc, ident)

            def load_w(dram, p, q):
                t32 = consts.tile([p, q], F32, tag=f"w32_{dram.name}")
                nc.sync.dma_start(out=t32, in_=dram[:])
                tr = consts.tile([p, q], F32R, tag=f"wr_{dram.name}")
                nc.vector.tensor_copy(tr[:], t32[:])
                return tr

            Wsr = load_w(Wsd, 128, 16)
            Wvar = load_w(Wvad, 128, 48)
            Wvbr = load_w(Wvbd, 64, 48)
            Wtar = load_w(Wtad, 128, 80)
            Wtbr = load_w(Wtbd, 32, 80)
            W1r = load_w(W1d, 128, 64)
            b1r = load_w(b1d, 1, 64)
            W2r = load_w(W2d, 64, 96)
            b2r = load_w(b2d, 1, 96)
            onesr = load_w(Od, 1, 512)
            Mt = consts.tile([128, 240], F32)
            nc.sync.dma_start(out=Mt, in_=Md[:])

            for m in range(N_MACRO):
                # ---------- load + transpose inputs ----------
                ing = [ingp.tile([128, 608], F32, tag=f"ing{g}", name=f"ing{g}") for g in range(4)]
                for g in range(4):
                    r0 = m * 512 + g * 128
                    nc.sync.dma_start(out=ing[g], in_=X[r0:r0 + 128, :])

                XT = stage.tile([128, 6, 512], F32R, tag="XT")
                for g in range(4):
                    pT1 = ps.tile([128, 384], F32, tag="pT1")
                    pT2 = ps.tile([128, 384], F32, tag="pT2")
                    nc.tensor.transpose(pT1[:, 0:128], ing[g][:, 0:128], ident[:])
                    nc.tensor.transpose(pT1[:, 128:256], ing[g][:, 128:256], ident[:])
                    nc.tensor.transpose(pT1[:, 256:384], ing[g][:, 256:384], ident[:])
                    nc.tensor.transpose(pT2[0:64, 0:128], ing[g][:, 384:448], ident[:])
                    nc.tensor.transpose(pT2[:, 128:256], ing[g][:, 448:576], ident[:])
                    nc.tensor.transpose(pT2[0:32, 256:384], ing[g][:, 576:608], ident[:])
                    c0 = g * 128
                    nc.scalar.copy(
                        XT[:, 0:3, c0:c0 + 128],
                        pT1[:].rearrange("p (b c) -> p b c", b=3))
                    nc.scalar.copy(
                        XT[:, 3:6, c0:c0 + 128],
                        pT2[:].rearrange("p (b c) -> p b c", b=3))

                # ---------- linear layers (f32r, weights stationary) ----------
                pS = ps.tile([16, 512], F32, tag="pS")
                pV = ps.tile([48, 512], F32, tag="pV")
                pT5 = ps.tile([80, 512], F32, tag="pT5")
                pH = ps.tile([64, 512], F32, tag="pH")
                pW = ps.tile([96, 512], F32, tag="pW")
                nc.tensor.matmul(pS[:], Wsr[:], XT[:, 1, :], start=True, stop=True)
                nc.tensor.matmul(pV[:], Wvar[:], XT[:, 2, :], start=True, stop=False)
                nc.tensor.matmul(pV[:], Wvbr[:], XT[0:64, 3, :], start=False, stop=True)
                nc.tensor.matmul(pT5[:], Wtar[:], XT[:, 4, :], start=True, stop=False)
                nc.tensor.matmul(pT5[:], Wtbr[:], XT[0:32, 5, :], start=False, stop=True)
                nc.tensor.matmul(pH[:], W1r[:], XT[:, 0, :], start=True, stop=False)
                nc.tensor.matmul(pH[:], b1r[:], onesr[:], start=False, stop=True)
                hT = stage.tile([64, 512], F32R, tag="hT")
                nc.scalar.activation(hT[:], pH[:], ACTF.Silu)
                nc.tensor.matmul(pW[:], W2r[:], hT[:], start=True, stop=False)
                nc.tensor.matmul(pW[:], b2r[:], onesr[:], start=False, stop=True)

                # feat-major copies to SBUF for the transpose-back
                sT = stage.tile([16, 512], F32, tag="sT")
                vT = stage.tile([48, 512], F32, tag="vT")
                tT = stage.tile([80, 512], F32, tag="tT")
                wT = stage.tile([96, 512], F32, tag="wT")
                nc.scalar.copy(sT[:], pS[:])
                nc.scalar.copy(vT[:], pV[:])
                nc.scalar.copy(tT[:], pT5[:])
                nc.scalar.copy(wT[:], pW[:])

                # ---------- transpose back to atom-major ----------
                am = tpp.tile([128, 4, 240], F32, tag="am")
                for g in range(4):
                    pb = ps.tile([128, 240], F32, tag="pb")
                    cg = slice(g * 128, g * 128 + 128)
                    nc.tensor.transpose(pb[:, 0:16], sT[:, cg], ident[0:16, 0:16])
                    nc.tensor.transpose(pb[:, 16:64], vT[:, cg], ident[0:48, 0:48])
                    nc.tensor.transpose(pb[:, 64:144], tT[:, cg], ident[0:80, 0:80])
                    nc.tensor.transpose(pb[:, 144:240], wT[:, cg], ident[0:96, 0:96])
                    nc.scalar.copy(am[:, g, :], pb[:])

                # ---------- tensor-product stage (atom-major) ----------
                s_ = am[:, :, 0:16]
                v_ = am[:, :, 16:64].rearrange("p g (i u) -> p g i u", i=3)
                t_ = am[:, :, 64:144].rearrange("p g (k u) -> p g k u", k=5)
                w0 = am[:, :, 144:160]
                w1 = am[:, :, 160:176].unsqueeze(2).broadcast_to([128, 4, 3, 16])
                w2 = am[:, :, 176:192].unsqueeze(2).broadcast_to([128, 4, 5, 16])
                wst = am[:, :, 192:208]
                w6 = am[:, :, 208:224].unsqueeze(2).broadcast_to([128, 4, 3, 16])
                w8 = am[:, :, 224:240].unsqueeze(2).broadcast_to([128, 4, 5, 16])

                sq = tpp.tile([128, 4, 144], F32, tag="sq")
                nc.scalar.activation(sq[:], am[:, :, 0:144], ACTF.Square)
                zb = tpp.tile([128, 4, 144], F32, tag="zb")
                nc.vector.tensor_mul(zb[:, :, 0:16], sq[:, :, 0:16], w0)
                nc.vector.tensor_mul(
                    zb[:, :, 16:64].rearrange("p g (i u) -> p g i u", i=3),
                    sq[:, :, 16:64].rearrange("p g (i u) -> p g i u", i=3), w1)
                nc.vector.tensor_mul(
                    zb[:, :, 64:144].rearrange("p g (k u) -> p g k u", k=5),
                    sq[:, :, 64:144].rearrange("p g (k u) -> p g k u", k=5), w2)

                fb = tpp.tile([128, 4, 40], F32, tag="fb")
                nc.vector.tensor_reduce(fb[:, :, 0:1], zb[:], axis=AX, op=ADD)

                qb = tpp.tile([128, 4, 16], F32, tag="qb")
                nc.vector.tensor_mul(qb[:], s_, am[:, :, 192:208])
                spb = tpp.tile([128, 4, 5, 16], F32, tag="spb")
                nc.vector.tensor_mul(
                    spb[:], qb[:].unsqueeze(2).broadcast_to([128, 4, 5, 16]), t_)
                nc.vector.tensor_reduce(fb[:, :, 1:6], spb[:], axis=AX, op=ADD)

                v6b = tpp.tile([128, 4, 3, 16], F32, tag="v6b")
                nc.vector.tensor_mul(v6b[:], v_, w6)
                vvb = tpp.tile([128, 4, 9, 16], F32, tag="vvb")
                for g in range(4):
                    nc.vector.tensor_mul(
                        vvb[:, g, :, :].rearrange("p (i j) u -> p i j u", i=3),
                        v6b[:, g, :, :].unsqueeze(2).broadcast_to([128, 3, 3, 16]),
                        v_[:, g, :, :].unsqueeze(1).broadcast_to([128, 3, 3, 16]))
                nc.vector.tensor_reduce(fb[:, :, 6:15], vvb[:], axis=AX, op=ADD)

                t8b = tpp.tile([128, 4, 5, 16], F32, tag="t8b")
                nc.vector.tensor_mul(t8b[:], t_, w8)
                ttb = tpp.tile([128, 4, 25, 16], F32, tag="ttb")
                for g in range(4):
                    nc.vector.tensor_mul(
                        ttb[:, g, :, :].rearrange("p (i j) u -> p i j u", i=5),
                        t8b[:, g, :, :].unsqueeze(2).broadcast_to([128, 5, 5, 16]),
                        t_[:, g, :, :].unsqueeze(1).broadcast_to([128, 5, 5, 16]))
                nc.vector.tensor_reduce(fb[:, :, 15:40], ttb[:], axis=AX, op=ADD)

                # ---------- final map F[40] @ M -> res6 -> res9 ----------
                rb = tpp.tile([128, 4, 6, 40], F32, tag="rb")
                Mv = Mt[:].rearrange("p (ab q) -> p ab q", ab=6)
                for g in range(4):
                    nc.vector.tensor_mul(
                        rb[:, g, :, :],
                        fb[:, g, :].unsqueeze(1).broadcast_to([128, 6, 40]), Mv)
                r6 = tpp.tile([128, 4, 6], F32, tag="r6")
                nc.vector.tensor_reduce(r6[:], rb[:], axis=AX, op=ADD)

                r9 = tpp.tile([128, 4, 9], F32, tag="r9")
                # diag xx,yy,zz -> cols 0,4,8
                nc.vector.tensor_copy(
                    bass_ap_strided(r9, 0, 4, 3), bass_ap_strided(r6, 0, 1, 3))
                # xy -> cols 1,3 ; xz -> cols 2,6 ; yz -> cols 5,7
                nc.vector.tensor_copy(bass_ap_strided(r9, 1, 2, 2), bass_ap_strided(r6, 3, 0, 2))
                nc.vector.tensor_copy(bass_ap_strided(r9, 2, 4, 2), bass_ap_strided(r6, 4, 0, 2))
                nc.vector.tensor_copy(bass_ap_strided(r9, 5, 2, 2), bass_ap_strided(r6, 5, 0, 2))

                yv = Y[m * 512:(m + 1) * 512, :].rearrange("(g p) x -> p g x", p=128)
                nc.sync.dma_start(out=yv, in_=r9[:])

    nc.compile()
    return nc


def bass_ap_strided(tile_, base, step, cnt):
    """view [128, 4, cnt] of a [128, 4, W] tile picking cols base, base+step*k."""
    ap = tile_[:]
    W = ap.shape[2]
    from concourse.ap import AP
    return AP(ap.tensor, ap.offset + base, [list(ap.ap[0]), [W, 4], [step, cnt]])


def kernel(**inputs):
    xs = np.ascontiguousarray(np.asarray(inputs["x_scalar"], np.float32))
    xp = np.ascontiguousarray(np.asarray(inputs["x_spherical"], np.float32))
    prep = _host_prep(
        np.asarray(inputs["W_lin0"], np.float32), np.asarray(inputs["W_lin1"], np.float32),
        np.asarray(inputs["W_lin2"], np.float32), np.asarray(inputs["W_mlp1"], np.float32),
        np.asarray(inputs["b_mlp1"], np.float32), np.asarray(inputs["W_mlp2"], np.float32),
        np.asarray(inputs["b_mlp2"], np.float32))

    n = xs.shape[0]
    Xall = np.empty((N_PAD, 608), np.float32)
    Xall[:n, 0:128] = xs
    Xall[:n, 128:608] = xp
    Xall[n:] = 0.0

    if _NC_CACHE[0] is None:
        _NC_CACHE[0] = _build_nc()
    nc = _NC_CACHE[0]

    from concourse.bass_utils import run_bass_kernel_spmd
    in_maps = []
    for c in range(N_CORES):
        im = dict(prep)
        im["X"] = Xall[c * A_CORE:(c + 1) * A_CORE]
        in_maps.append(im)
    t0 = time.time()
    res = run_bass_kernel_spmd(nc, in_maps, list(range(N_CORES)))
    kernel._last_exec_s = time.time() - t0
    out = np.concatenate([res.results[c]["Y"] for c in range(N_CORES)], axis=0)
    return out[:n].reshape(n, 3, 3).astype(np.float32)


# revision 7
# speedup vs baseline: 1.2675x; 1.2675x over previous
"""TRN2 Bass kernel for nn_CSCOut_61503931678830 (e3nn-style gated tensor product).

Self-contained: computes Wigner-3j constants locally, shards the atom dim
across 8 NeuronCores (pure data parallel), and runs one Bass/Tile kernel
per core via run_bass_kernel_spmd.

Math (validated vs reference in fp32):
  paths 3,4 (antisymmetric C111/C221 with x1==x2) are identically zero.
  res[n] = F[n] @ M where F = [out0(1), B5(5), G9(9), H25(25)] and all
  path norms / C-diagonals / Q-basis / axis permutation fold into M and
  the preprocessed weights.
"""
import math
import os
import sys
import time

sys.path.insert(0, "/opt/trn_rl_repo")

import numpy as np

# ----------------------------------------------------------------------------
# Wigner-3j constants (identical math to the reference, self-contained)
# ----------------------------------------------------------------------------

def _cg(j1, m1, j2, m2, j3, m3):
    if m3 != m1 + m2:
        return 0.0
    f = math.factorial
    vmin = max(-j1 + j2 + m3, -j1 + m1, 0)
    vmax = min(j2 + j3 + m1, j3 - j1 + j2, j3 + m3)
    pref = math.sqrt((2 * j3 + 1) * f(j3 + j1 - j2) * f(j3 - j1 + j2) * f(j1 + j2 - j3) / f(j1 + j2 + j3 + 1))
    pref *= math.sqrt(f(j3 + m3) * f(j3 - m3) / (f(j1 + m1) * f(j1 - m1) * f(j2 + m2) * f(j2 - m2)))
    s = 0.0
    for v in range(vmin, vmax + 1):
        s += (-1.0) ** (v + j2 + m2) * f(j2 + j3 + m1 - v) * f(j1 - m1 + v) / (
            f(v) * f(j3 - j1 + j2 - v) * f(j3 + m3 - v) * f(v + j1 - j2 - m3))
    return pref * s


def _r2c(l):
    q = np.zeros((2 * l + 1, 2 * l + 1), dtype=np.complex128)
    for m in range(-l, 0):
        q[l + m, l + abs(m)] = 1 / 2 ** 0.5
        q[l + m, l - abs(m)] = -1j / 2 ** 0.5
    q[l, l] = 1.0
    for m in range(1, l + 1):
        q[l + m, l + abs(m)] = (-1) ** m / 2 ** 0.5
        q[l + m, l - abs(m)] = 1j * (-1) ** m / 2 ** 0.5
    return (-1j) ** l * q


def _w3j(l1, l2, l3):
    C = np.zeros((2 * l1 + 1, 2 * l2 + 1, 2 * l3 + 1))
    for m1 in range(-l1, l1 + 1):
        for m2 in range(-l2, l2 + 1):
            m3 = m1 + m2
            if abs(m3) <= l3:
                C[l1 + m1, l2 + m2, l3 + m3] = _cg(l1, m1, l2, m2, l3, m3)
    C = C / math.sqrt(2 * l3 + 1)
    Cr = np.einsum('ij,kl,mn,ikm->jln', _r2c(l1), _r2c(l2), _r2c(l3), C.astype(np.complex128))
    Cr = Cr.imag if np.linalg.norm(Cr.imag) > np.linalg.norm(Cr.real) else Cr.real
    return (Cr / np.linalg.norm(Cr)).astype(np.float32)


_C110, _C220 = _w3j(1, 1, 0), _w3j(2, 2, 0)
_C022, _C112, _C222 = _w3j(0, 2, 2), _w3j(1, 1, 2), _w3j(2, 2, 2)
_Q = np.concatenate([math.sqrt(2 * l + 1) * np.transpose(_w3j(1, 1, l), (2, 0, 1)) for l in (0, 1, 2)], axis=0)
_N0, _N2 = (1.0 / 48.0) ** 0.5, (5.0 / 64.0) ** 0.5
_idx = [2, 0, 1]
_Qp = _Q[:, _idx, :][:, :, _idx]
_c110 = float(_C110[0, 0, 0])
_c220 = float(_C220[0, 0, 0])
_sig = np.array([_C022[0, k, k] for k in range(5)], np.float64)

N_CORES = 8
N_FULL = 200_000
A_CORE = 25_088          # 49 * 512, padded per-core atom count
N_PAD = N_CORES * A_CORE
N_MACRO = A_CORE // 512  # 49


def _host_prep(W_lin0, W_lin1, W_lin2, W_mlp1, b_mlp1, W_mlp2, b_mlp2):
    Ws = (W_lin0 / math.sqrt(128.0)).astype(np.float32)          # [128,16]
    Wv = np.zeros((192, 48), np.float32)
    for i in range(3):
        Wv[i::3, i * 16:(i + 1) * 16] = W_lin1 / math.sqrt(64.0)
    Wt = np.zeros((160, 80), np.float32)
    for k in range(5):
        Wt[k::5, k * 16:(k + 1) * 16] = W_lin2 / math.sqrt(32.0)
    W2c = np.zeros((64, 96), np.float32)
    b2c = np.zeros((1, 96), np.float32)
    scales = [_N0, _N0 * _c110, _N0 * _c220, None, _N2, _N2]
    blocks = [0, 1, 2, None, 6, 8]
    for j, (p, a) in enumerate(zip(blocks, scales)):
        if p is None:
            W2c[:, j * 16:(j + 1) * 16] = _N2 * (W_mlp2[:, 80:96] + W_mlp2[:, 112:128])
            b2c[0, j * 16:(j + 1) * 16] = _N2 * (b_mlp2[80:96] + b_mlp2[112:128])
        else:
            W2c[:, j * 16:(j + 1) * 16] = a * W_mlp2[:, p * 16:(p + 1) * 16]
            b2c[0, j * 16:(j + 1) * 16] = a * b_mlp2[p * 16:(p + 1) * 16]
    # final map M [40, 9] -> symmetric 6 cols [xx,yy,zz,xy,xz,yz]
    M9 = np.zeros((40, 9), np.float64)
    M9[0] = _Qp[0].reshape(9)
    for k in range(5):
        M9[1 + k] = _sig[k] * _Qp[4 + k].reshape(9)
    for i in range(3):
        for j in range(3):
            for k in range(5):
                M9[6 + i * 3 + j] += _C112[i, j, k] * _Qp[4 + k].reshape(9)
    for i in range(5):
        for j in range(5):
            for k in range(5):
                M9[15 + i * 5 + j] += _C222[i, j, k] * _Qp[4 + k].reshape(9)
    M9 = M9.reshape(40, 3, 3)
    M6 = np.stack([M9[:, 0, 0], M9[:, 1, 1], M9[:, 2, 2],
                   M9[:, 0, 1], M9[:, 0, 2], M9[:, 1, 2]], axis=1)  # [40, 6]
    Mrep = np.tile(M6.T.reshape(1, 240).astype(np.float32), (128, 1))  # [128, 6*40] ab-major
    ones = np.ones((1, 512), np.float32)
    return dict(Ws=Ws, Wva=Wv[:128].copy(), Wvb=Wv[128:].copy(),
                Wta=Wt[:128].copy(), Wtb=Wt[128:].copy(),
                W1=W_mlp1.astype(np.float32), b1=b_mlp1.reshape(1, 64).astype(np.float32),
                W2=W2c, b2=b2c, Mrep=Mrep, ones=ones)


_NC_CACHE = [None]


def _build_nc(n_macro=None):
    import concourse.bacc as bacc
    import concourse.tile as tile
    from concourse import mybir
    from concourse.masks import make_identity

    F32 = mybir.dt.float32
    F32R = mybir.dt.float32r
    AX = mybir.AxisListType.X
    ADD = mybir.AluOpType.add
    ACTF = mybir.ActivationFunctionType

    nc = bacc.Bacc("TRN2", target_bir_lowering=False, debug=False)
    X = nc.declare_dram_parameter("X", [A_CORE, 608], F32, isOutput=False)
    Wsd = nc.declare_dram_parameter("Ws", [128, 16], F32, isOutput=False)
    Wvad = nc.declare_dram_parameter("Wva", [128, 48], F32, isOutput=False)
    Wvbd = nc.declare_dram_parameter("Wvb", [64, 48], F32, isOutput=False)
    Wtad = nc.declare_dram_parameter("Wta", [128, 80], F32, isOutput=False)
    Wtbd = nc.declare_dram_parameter("Wtb", [32, 80], F32, isOutput=False)
    W1d = nc.declare_dram_parameter("W1", [128, 64], F32, isOutput=False)
    b1d = nc.declare_dram_parameter("b1", [1, 64], F32, isOutput=False)
    W2d = nc.declare_dram_parameter("W2", [64, 96], F32, isOutput=False)
    b2d = nc.declare_dram_parameter("b2", [1, 96], F32, isOutput=False)
    Md = nc.declare_dram_parameter("Mrep", [128, 240], F32, isOutput=False)
    Od = nc.declare_dram_parameter("ones", [1, 512], F32, isOutput=False)
    Y = nc.declare_dram_parameter("Y", [A_CORE, 9], F32, isOutput=True)

    with tile.TileContext(nc) as tc:
        with (
            tc.tile_pool(name="consts", bufs=1) as consts,
            tc.tile_pool(name="stage", bufs=2) as stage,
            tc.tile_pool(name="ing", bufs=2) as ingp,
            tc.tile_pool(name="tp", bufs=2) as tpp,
            tc.tile_pool(name="ps", bufs=1, space="PSUM") as ps,
        ):
            # ---- one-time weight staging (fp32 load -> f32r rounded) ----
            ident = consts.tile([128, 128], F32)
            make_identity(nc, ident)

            def load_w(dram, p, q):
                t32 = consts.tile([p, q], F32, tag=f"w32_{dram.name}")
                nc.sync.dma_start(out=t32, in_=dram[:])
                tr = consts.tile([p, q], F32R, tag=f"wr_{dram.name}")
                nc.vector.tensor_copy(tr[:], t32[:])
                return tr

            Wsr = load_w(Wsd, 128, 16)
            Wvar = load_w(Wvad, 128, 48)
            Wvbr = load_w(Wvbd, 64, 48)
            Wtar = load_w(Wtad, 128, 80)
            Wtbr = load_w(Wtbd, 32, 80)
            W1r = load_w(W1d, 128, 64)
            b1r = load_w(b1d, 1, 64)
            W2r = load_w(W2d, 64, 96)
            b2r = load_w(b2d, 1, 96)
            onesr = load_w(Od, 1, 512)
            Mt = consts.tile([128, 240], F32)
            nc.sync.dma_start(out=Mt, in_=Md[:])

            for m in range(n_macro if n_macro is not None else N_MACRO):
                # ---------- load + transpose inputs ----------
                ing = [ingp.tile([128, 608], F32, tag=f"ing{g}", name=f"ing{g}") for g in range(4)]
                for g in range(4):
                    r0 = m * 512 + g * 128
                    nc.sync.dma_start(out=ing[g], in_=X[r0:r0 + 128, :])

                XT = stage.tile([128, 6, 512], F32R, tag="XT")
                for g in range(4):
                    pT1 = ps.tile([128, 384], F32, tag="pT1")
                    pT2 = ps.tile([128, 384], F32, tag="pT2")
                    nc.tensor.transpose(pT1[:, 0:128], ing[g][:, 0:128], ident[:])
                    nc.tensor.transpose(pT1[:, 128:256], ing[g][:, 128:256], ident[:])
                    nc.tensor.transpose(pT1[:, 256:384], ing[g][:, 256:384], ident[:])
                    nc.tensor.transpose(pT2[0:64, 0:128], ing[g][:, 384:448], ident[:])
                    nc.tensor.transpose(pT2[:, 128:256], ing[g][:, 448:576], ident[:])
                    nc.tensor.transpose(pT2[0:32, 256:384], ing[g][:, 576:608], ident[:])
                    c0 = g * 128
                    nc.scalar.copy(
                        XT[:, 0:3, c0:c0 + 128],
                        pT1[:].rearrange("p (b c) -> p b c", b=3))
                    nc.scalar.copy(
                        XT[:, 3:6, c0:c0 + 128],
                        pT2[:].rearrange("p (b c) -> p b c", b=3))

                # ---------- linear layers (f32r, weights stationary) ----------
                pS = ps.tile([16, 512], F32, tag="pS")
                pV = ps.tile([48, 512], F32, tag="pV")
                pT5 = ps.tile([80, 512], F32, tag="pT5")
                pH = ps.tile([64, 512], F32, tag="pH")
                pW = ps.tile([96, 512], F32, tag="pW")
                nc.tensor.matmul(pS[:], Wsr[:], XT[:, 1, :], start=True, stop=True)
                nc.tensor.matmul(pV[:], Wvar[:], XT[:, 2, :], start=True, stop=False)
                nc.tensor.matmul(pV[:], Wvbr[:], XT[0:64, 3, :], start=False, stop=True)
                nc.tensor.matmul(pT5[:], Wtar[:], XT[:, 4, :], start=True, stop=False)
                nc.tensor.matmul(pT5[:], Wtbr[:], XT[0:32, 5, :], start=False, stop=True)
                nc.tensor.matmul(pH[:], W1r[:], XT[:, 0, :], start=True, stop=False)
                nc.tensor.matmul(pH[:], b1r[:], onesr[:], start=False, stop=True)
                hT = stage.tile([64, 512], F32R, tag="hT")
                nc.scalar.activation(hT[:], pH[:], ACTF.Silu)
                nc.tensor.matmul(pW[:], W2r[:], hT[:], start=True, stop=False)
                nc.tensor.matmul(pW[:], b2r[:], onesr[:], start=False, stop=True)

                # feat-major copies to SBUF for the transpose-back
                sT = stage.tile([16, 512], F32, tag="sT")
                vT = stage.tile([48, 512], F32, tag="vT")
                tT = stage.tile([80, 512], F32, tag="tT")
                wT = stage.tile([96, 512], F32, tag="wT")
                nc.scalar.copy(sT[:], pS[:])
                nc.scalar.copy(vT[:], pV[:])
                nc.scalar.copy(tT[:], pT5[:])
                nc.scalar.copy(wT[:], pW[:])

                # ---------- transpose back to atom-major ----------
                am = tpp.tile([128, 4, 240], F32, tag="am")
                for g in range(4):
                    pb = ps.tile([128, 240], F32, tag="pb")
                    cg = slice(g * 128, g * 128 + 128)
                    nc.tensor.transpose(pb[:, 0:16], sT[:, cg], ident[0:16, 0:16])
                    nc.tensor.transpose(pb[:, 16:64], vT[:, cg], ident[0:48, 0:48])
                    nc.tensor.transpose(pb[:, 64:144], tT[:, cg], ident[0:80, 0:80])
                    nc.tensor.transpose(pb[:, 144:240], wT[:, cg], ident[0:96, 0:96])
                    nc.scalar.copy(am[:, g, :], pb[:])

                # ---------- tensor-product stage (atom-major) ----------
                s_ = am[:, :, 0:16]
                v_ = am[:, :, 16:64].rearrange("p g (i u) -> p g i u", i=3)
                t_ = am[:, :, 64:144].rearrange("p g (k u) -> p g k u", k=5)
                w0 = am[:, :, 144:160]
                w1 = am[:, :, 160:176].unsqueeze(2).broadcast_to([128, 4, 3, 16])
                w2 = am[:, :, 176:192].unsqueeze(2).broadcast_to([128, 4, 5, 16])
                wst = am[:, :, 192:208]
                w6 = am[:, :, 208:224].unsqueeze(2).broadcast_to([128, 4, 3, 16])
                w8 = am[:, :, 224:240].unsqueeze(2).broadcast_to([128, 4, 5, 16])

                sq = tpp.tile([128, 4, 144], F32, tag="sq")
                nc.gpsimd.tensor_mul(sq[:], am[:, :, 0:144], am[:, :, 0:144])
                zb = tpp.tile([128, 4, 144], F32, tag="zb")
                nc.vector.tensor_mul(zb[:, :, 0:16], sq[:, :, 0:16], w0)
                nc.vector.tensor_mul(
                    zb[:, :, 16:64].rearrange("p g (i u) -> p g i u", i=3),
                    sq[:, :, 16:64].rearrange("p g (i u) -> p g i u", i=3), w1)
                nc.vector.tensor_mul(
                    zb[:, :, 64:144].rearrange("p g (k u) -> p g k u", k=5),
                    sq[:, :, 64:144].rearrange("p g (k u) -> p g k u", k=5), w2)

                fb = tpp.tile([128, 4, 40], F32, tag="fb")
                nc.vector.tensor_reduce(fb[:, :, 0:1], zb[:], axis=AX, op=ADD)

                qb = tpp.tile([128, 4, 16], F32, tag="qb")
                nc.vector.tensor_mul(qb[:], s_, am[:, :, 192:208])
                spb = tpp.tile([128, 4, 5, 16], F32, tag="spb")
                nc.vector.tensor_mul(
                    spb[:], qb[:].unsqueeze(2).broadcast_to([128, 4, 5, 16]), t_)
                nc.vector.tensor_reduce(fb[:, :, 1:6], spb[:], axis=AX, op=ADD)

                v6b = tpp.tile([128, 4, 3, 16], F32, tag="v6b")
                nc.gpsimd.tensor_mul(v6b[:], v_, w6)
                vvb = tpp.tile([128, 4, 9, 16], F32, tag="vvb")
                for g in range(4):
                    nc.gpsimd.tensor_mul(
                        vvb[:, g, :, :].rearrange("p (i j) u -> p i j u", i=3),
                        v6b[:, g, :, :].unsqueeze(2).broadcast_to([128, 3, 3, 16]),
                        v_[:, g, :, :].unsqueeze(1).broadcast_to([128, 3, 3, 16]))
                nc.vector.tensor_reduce(fb[:, :, 6:15], vvb[:], axis=AX, op=ADD)

                t8b = tpp.tile([128, 4, 5, 16], F32, tag="t8b")
                nc.gpsimd.tensor_mul(t8b[:], t_, w8)
                ttb = tpp.tile([128, 4, 25, 16], F32, tag="ttb")
                for g in range(4):
                    nc.gpsimd.tensor_mul(
                        ttb[:, g, :, :].rearrange("p (i j) u -> p i j u", i=5),
                        t8b[:, g, :, :].unsqueeze(2).broadcast_to([128, 5, 5, 16]),
                        t_[:, g, :, :].unsqueeze(1).broadcast_to([128, 5, 5, 16]))
                nc.vector.tensor_reduce(fb[:, :, 15:40], ttb[:], axis=AX, op=ADD)

                # ---------- final map F[40] @ M -> res6 -> res9 ----------
                rb = tpp.tile([128, 4, 6, 40], F32, tag="rb")
                Mv = Mt[:].rearrange("p (ab q) -> p ab q", ab=6)
                for g in range(4):
                    nc.vector.tensor_mul(
                        rb[:, g, :, :],
                        fb[:, g, :].unsqueeze(1).broadcast_to([128, 6, 40]), Mv)
                r6 = tpp.tile([128, 4, 6], F32, tag="r6")
                nc.vector.tensor_reduce(r6[:], rb[:], axis=AX, op=ADD)

                r9 = tpp.tile([128, 4, 9], F32, tag="r9")
                # diag xx,yy,zz -> cols 0,4,8
                nc.vector.tensor_copy(
                    bass_ap_strided(r9, 0, 4, 3), bass_ap_strided(r6, 0, 1, 3))
                # xy -> cols 1,3 ; xz -> cols 2,6 ; yz -> cols 5,7
                nc.vector.tensor_copy(bass_ap_strided(r9, 1, 2, 2), bass_ap_strided(r6, 3, 0, 2))
                nc.vector.tensor_copy(bass_ap_strided(r9, 2, 4, 2), bass_ap_strided(r6, 4, 0, 2))
                nc.vector.tensor_copy(bass_ap_strided(r9, 5, 2, 2), bass_ap_strided(r6, 5, 0, 2))

                yv = Y[m * 512:(m + 1) * 512, :].rearrange("(g p) x -> p g x", p=128)
                nc.sync.dma_start(out=yv, in_=r9[:])

    nc.compile()
    return nc


def bass_ap_strided(tile_, base, step, cnt):
    """view [128, 4, cnt] of a [128, 4, W] tile picking cols base, base+step*k."""
    ap = tile_[:]
    W = ap.shape[2]
    from concourse.ap import AP
    return AP(ap.tensor, ap.offset + base, [list(ap.ap[0]), [W, 4], [step, cnt]])


def kernel(**inputs):
    xs = np.ascontiguousarray(np.asarray(inputs["x_scalar"], np.float32))
    xp = np.ascontiguousarray(np.asarray(inputs["x_spherical"], np.float32))
    prep = _host_prep(
        np.asarray(inputs["W_lin0"], np.float32), np.asarray(inputs["W_lin1"], np.float32),
        np.asarray(inputs["W_lin2"], np.float32), np.asarray(inputs["W_mlp1"], np.float32),
        np.asarray(inputs["b_mlp1"], np.float32), np.asarray(inputs["W_mlp2"], np.float32),
        np.asarray(inputs["b_mlp2"], np.float32))

    n = xs.shape[0]
    Xall = np.empty((N_PAD, 608), np.float32)
    Xall[:n, 0:128] = xs
    Xall[:n, 128:608] = xp
    Xall[n:] = 0.0

    if _NC_CACHE[0] is None:
        _NC_CACHE[0] = _build_nc()
    nc = _NC_CACHE[0]

    from concourse.bass_utils import run_bass_kernel_spmd
    in_maps = []
    for c in range(N_CORES):
        im = dict(prep)
        im["X"] = Xall[c * A_CORE:(c + 1) * A_CORE]
        in_maps.append(im)
    t0 = time.time()
    res = run_bass_kernel_spmd(nc, in_maps, list(range(N_CORES)))
    kernel._last_exec_s = time.time() - t0
    out = np.concatenate([res.results[c]["Y"] for c in range(N_CORES)], axis=0)
    return out[:n].reshape(n, 3, 3).astype(np.float32)
